# revision 4
# baseline (speedup 1.0000x reference)
"""CenterLoss kernel for 8 TRN2 NeuronCores (raw Bass).

Computes mean_i ||x_i - center[labels_i]||^2 for x:[8192,128] f32,
center:[32000,128] f32, labels:[8192] int, via the decomposition

    sum ||x - g||^2 = sum x^2 - 2 sum x.g + sum g^2,   g_i = center[labels_i]

Strategy (data-parallel over the batch dim, per the sharding hint):
  - 8 cores, each takes a 1024-row shard of x/labels; the center table
    stays in HBM on every core and only the 1024 labeled rows are read,
    via SWDGE dma_gather in two pipelined pieces (5 + 3 chunks of 128
    rows), each prepared on the Q7 then trigger-fired so the transfer
    starts the moment its descriptor generation commits.
  - One packed idx DMA (wrapped int16 gather indices for both pieces +
    the replicated identity indices for the output scatter) so the Q7
    can start descriptor generation as early as possible.
  - Compute, one fused op per term: Act does Square-accumulate for
    sum x^2 (early, while gathers are in flight) and sum g^2 per piece;
    DVE scalar_tensor_tensor computes (-2x)*g with accum_out per piece.
    Each op lands in its own column of a [128,64]
    partial tile; no cross-engine combining on device.
  - Output via a prepared dma_scatter_add (identity indices) triggered
    once all five accumulating ops have signalled; the host sums the
    8 x 128 x 5 partials and divides by 8192 (the scalar all-reduce).
  - The framework's start/end all-engine barriers and drains are
    stripped post-build: every cross-engine data edge is ordered through
    DMA-completion or engine semaphores (Act's const-0 bias read is
    ordered behind the Pool-rooted gather-sem chain), so the barriers
    only add latency.

Validated exact (rel err ~1e-16 vs the fp32 reference recomputation)
across seeds on the axon execution path; TimelineSim 8419ns/core.

The kernel is self-contained: shapes are hardcoded below.
"""

import numpy as np

N, D, M = 8192, 128, 32000
NCORES = 8
NS = N // NCORES          # rows per core = 1024
C = NS // 128             # free-dim chunks per core = 8

# gather pieces: (chunk_start, chunk_end), all via SWDGE prepare+trigger
PIECES = ((0, 5), (5, 8))
# accumulating ops: (engine V/A, kind xx/xg/gg, chunk_start, chunk_end)
OPS = (
    ("A", "xx", 0, 8),
    ("V", "xg", 0, 5),
    ("V", "xg", 5, 8),
    ("A", "gg", 0, 5),
    ("A", "gg", 5, 8),
)
# packed idx tensor layout (int32 columns):
#   [0, 20)  piece-0 wrapped int16 idx (40 i16 cols)
#   [20, 32) piece-1 wrapped int16 idx (24 i16 cols)
#   [32, 36) scatter identity idx (8 i16 cols)
SW_COLS = {0: (0, 20), 1: (20, 12)}
SC2 = 32
IDXW = 36
OC = 64                   # scatter elem = 64 f32 = 256B (hardware minimum)

_CACHE: dict = {}


def _strip_barriers(nc):
    """Remove the framework's start/end all-engine barriers and drains.

    Every cross-engine dependency in this program flows through DMA or
    engine semaphores, so the barriers only serialize the launch/tail.
    """
    fn = nc.m.functions[0]
    for bb in fn.blocks:
        il = bb.instructions
        for inst in list(il):
            tn = type(inst).__name__
            if tn == "InstEventSemaphore" and inst.name.startswith("barrier_"):
                il.remove(inst)
            elif tn == "InstDrain":
                il.remove(inst)


def _build():
    import concourse.bacc as bacc
    import concourse.bass as bass
    import concourse.mybir as mybir

    nc = bacc.Bacc(
        "TRN2",
        target_bir_lowering=False,
        debug=False,
        enable_asserts=False,
        num_devices=NCORES,
    )
    f32 = mybir.dt.float32
    x_d = nc.dram_tensor("x", [NS, D], f32, kind="ExternalInput")
    c_d = nc.dram_tensor("center", [M, D], f32, kind="ExternalInput")
    i_d = nc.dram_tensor("idx", [128, IDXW], mybir.dt.int32, kind="ExternalInput")
    o_d = nc.dram_tensor("out", [128, OC], f32, kind="ExternalOutput")
    nred = len(OPS)

    with (
        nc.sbuf_tensor("x_t", [128, C, D], f32) as x_t,
        nc.sbuf_tensor("g_t", [128, C, D], f32) as g_t,
        nc.sbuf_tensor("tmpv", [128, C, D], f32) as tmpv,
        nc.sbuf_tensor("tmpa", [128, C, D], f32) as tmpa,
        nc.sbuf_tensor("idx_t", [128, IDXW], mybir.dt.int32) as idx_t,
        nc.sbuf_tensor("obuf", [128, OC], f32) as obuf,
        nc.sbuf_tensor("zbias", [128, 1], f32) as zbias,
        nc.sbuf_tensor("zdum", [128, 1], f32) as zdum,
        nc.semaphore("s_idx") as s_idx,
        nc.semaphore("s_z") as s_z,
        nc.semaphore("s_x") as s_x,
        nc.semaphore("s_g0") as s_g0,
        nc.semaphore("s_g1") as s_g1,
        nc.semaphore("s_prep") as s_prep,
        nc.semaphore("s_red") as s_red,
        nc.semaphore("s_out") as s_out,
        nc.Block() as block,
    ):
        s_g = [s_g0, s_g1]

        @block.sync
        def _(sync: "bass.BassSync"):
            # idx first: it gates the Q7 descriptor generation
            sync.dma_start(idx_t[:], i_d.ap()).then_inc(s_idx, 16)
            # x as one contiguous DMA; slot (q, c) = row q*C + c
            x_src = x_d.ap().rearrange("(q c) d -> q c d", q=128)
            sync.dma_start(x_t[:], x_src).then_inc(s_x, 16)

        @block.gpsimd
        def _(gpsimd: "bass.BassGpSimd"):
            gpsimd.wait_ge(s_idx, 16)
            for i, (c0, c1) in enumerate(PIECES):
                sc, ncol32 = SW_COLS[i]
                n_idx = (c1 - c0) * 128
                idx16 = idx_t[:, sc:sc + ncol32].bitcast(mybir.dt.int16)
                gpsimd.dma_gather(
                    g_t[:, c0:c1, :], c_d.ap(), idx16,
                    n_idx, n_idx, D,
                    prepare_only=True, sem=s_g[i],
                ).then_inc(s_prep, 1)
                gpsimd.wait_ge(s_prep, i + 1)
                gpsimd.trigger_dma(count=1)
            gpsimd.dma_scatter_add(
                o_d.ap(),
                obuf[:].rearrange("q (a e) -> q a e", a=1),
                idx_t[:, SC2:SC2 + 4].bitcast(mybir.dt.int16),
                128, 128, OC,
                prepare_only=True, sem=s_out,
            ).then_inc(s_prep, 1)
            gpsimd.wait_ge(s_prep, len(PIECES) + 1)
            gpsimd.wait_ge(s_red, nred)
            gpsimd.trigger_dma(count=1)

        def emit_op(eng, is_act, kind, c0, c1, colidx, tmp):
            xs = x_t[:, c0:c1, :]
            gs = g_t[:, c0:c1, :]
            if kind in ("xx", "xg"):
                eng.wait_ge(s_x, 16)
            if kind in ("gg", "xg"):
                for i, (p0, p1) in enumerate(PIECES):
                    if c0 < p1 and c1 > p0:
                        eng.wait_ge(s_g[i], 16)
            if is_act:
                src = gs if kind == "gg" else xs
                eng.activation(
                    tmp[:, c0:c1, :], src,
                    mybir.ActivationFunctionType.Square,
                    bias=zbias[:],
                    accum_out=obuf[:, colidx:colidx + 1],
                ).then_inc(s_red, 1)
            else:
                if kind == "xg":
                    in0, in1, scal = xs, gs, -2.0
                elif kind == "gg":
                    in0, in1, scal = gs, gs, 1.0
                else:
                    in0, in1, scal = xs, xs, 1.0
                eng.scalar_tensor_tensor(
                    tmp[:, c0:c1, :], in0, scal, in1,
                    mybir.AluOpType.mult, mybir.AluOpType.mult,
                    accum_out=obuf[:, colidx:colidx + 1],
                ).then_inc(s_red, 1)

        @block.vector
        def _(vector: "bass.BassVector"):
            vector.memset(zbias[:], 0.0).then_inc(s_z, 1)
            vector.memset(obuf[:], 0.0).then_inc(s_z, 1)
            for i, (e, kind, c0, c1) in enumerate(OPS):
                if e == "V":
                    emit_op(vector, False, kind, c0, c1, i, tmpv)

        @block.scalar
        def _(scalar: "bass.BassScalar"):
            scalar.wait_ge(s_z, 2)
            # tiny dummy activation: forces the Square table load off the
            # critical path (it would otherwise run right before the first
            # real activation, after its data waits)
            scalar.activation(
                zdum[:], zbias[:],
                mybir.ActivationFunctionType.Square, bias=zbias[:])
            for i, (e, kind, c0, c1) in enumerate(OPS):
                if e == "A":
                    emit_op(scalar, True, kind, c0, c1, i, tmpa)

    _strip_barriers(nc)
    nc.compile()
    return nc


def _get_nc():
    if "nc" not in _CACHE:
        _CACHE["nc"] = _build()
    return _CACHE["nc"]


def make_in_maps(inputs: np.ndarray, center: np.ndarray, labels: np.ndarray):
    """Shard full inputs into per-core input maps."""
    x = np.ascontiguousarray(np.asarray(inputs, dtype=np.float32))
    cen = np.ascontiguousarray(np.asarray(center, dtype=np.float32))
    lab = np.asarray(labels).astype(np.int32)
    # scatter identity, wrapped in 16 partitions and replicated x8
    wi = np.arange(128, dtype=np.int16).reshape(8, 16).T          # [16, 8]
    wi128 = np.ascontiguousarray(np.tile(wi, (8, 1))).view(np.int32)
    in_maps = []
    for k in range(NCORES):
        lab_k = lab[k * NS:(k + 1) * NS]
        offs = lab_k.reshape(128, C)          # offs[q, c] = label of row q*C+c
        idx = np.zeros((128, IDXW), dtype=np.int32)
        for i, (c0, c1) in enumerate(PIECES):
            sc, ncol32 = SW_COLS[i]
            n = (c1 - c0) * 128
            # gather element j -> slot (q=j%128, c=c0+j//128); wrapped int16
            # layout: element j at [j%16 (+16g replicas), j//16]
            j = np.arange(n)
            vals = offs[j % 128, c0 + j // 128].astype(np.int16)
            w = vals.reshape(-1, 16).T
            idx[:, sc:sc + ncol32] = np.ascontiguousarray(
                np.tile(w, (8, 1))).view(np.int32)
        idx[:, SC2:SC2 + 4] = wi128
        in_maps.append({
            "x": np.ascontiguousarray(x[k * NS:(k + 1) * NS]),
            "center": cen,
            "idx": idx,
        })
    return in_maps


def _run(in_maps):
    from concourse.bass_utils import run_bass_kernel_spmd

    nc = _get_nc()
    return run_bass_kernel_spmd(nc, in_maps, core_ids=list(range(NCORES)))


def kernel(inputs: np.ndarray, center: np.ndarray, labels: np.ndarray) -> np.ndarray:
    in_maps = make_in_maps(inputs, center, labels)
    res = _run(in_maps)
    # unshard: sum the per-core per-partition column partials, then the mean
    total = np.sum(
        np.stack([r["out"][:, :len(OPS)].astype(np.float32) for r in res.results]),
        dtype=np.float32,
    )
    return np.asarray(np.float32(total / np.float32(N)), dtype=np.float32)


if __name__ == "__main__":
    rng = np.random.default_rng(0)
    x = rng.standard_normal((N, D), dtype=np.float32)
    cen = rng.standard_normal((M, D), dtype=np.float32)
    lab = rng.integers(0, M, size=(N,), dtype=np.int64)
    got = kernel(x, cen, lab)
    sel = cen[lab]
    ref = np.mean(np.clip(np.sum((x - sel) ** 2, axis=1), 1e-12, 1e12))
    print("got", got, "ref", ref, "rel", abs(got - ref) / abs(ref))


# revision 5
# speedup vs baseline: 1.0084x; 1.0084x over previous
"""CenterLoss kernel for 8 TRN2 NeuronCores (raw Bass).

Computes mean_i ||x_i - center[labels_i]||^2 for x:[8192,128] f32,
center:[32000,128] f32, labels:[8192] int, via the decomposition

    sum ||x - g||^2 = sum x^2 - 2 sum x.g + sum g^2,   g_i = center[labels_i]

Strategy (data-parallel over the batch dim, per the sharding hint):
  - 8 cores, each takes a 1024-row shard of x/labels; the center table
    stays in HBM on every core and only the 1024 labeled rows are read,
    via SWDGE dma_gather in two pipelined pieces (5 + 3 chunks of 128
    rows), each prepared on the Q7 then trigger-fired so the transfer
    starts the moment its descriptor generation commits.
  - One packed idx DMA (wrapped int16 gather indices for both pieces +
    the replicated identity indices for the output scatter) so the Q7
    can start descriptor generation as early as possible.
  - Compute, one fused op per term: Act does Square-accumulate for
    sum x^2 (early, while gathers are in flight) and sum g^2 per piece;
    DVE scalar_tensor_tensor computes (-2x)*g with accum_out per piece.
    Each op lands in its own column of a [128,64]
    partial tile; no cross-engine combining on device.
  - Output via a prepared dma_scatter_add (identity indices) triggered
    once all six accumulating ops have signalled; the host sums the
    8 x 128 x 6 partials and divides by 8192 (the scalar all-reduce).
  - The framework's start/end all-engine barriers and drains are
    stripped post-build: every cross-engine data edge is ordered through
    DMA-completion or engine semaphores (Act's const-0 bias read is
    ordered behind the Pool-rooted gather-sem chain), so the barriers
    only add latency.

Validated exact (rel err ~1e-16 vs the fp32 reference recomputation)
across seeds on the axon execution path; TimelineSim 8349ns/core.

The kernel is self-contained: shapes are hardcoded below.
"""

import numpy as np

N, D, M = 8192, 128, 32000
NCORES = 8
NS = N // NCORES          # rows per core = 1024
C = NS // 128             # free-dim chunks per core = 8

# gather pieces: (chunk_start, chunk_end), all via SWDGE prepare+trigger
PIECES = ((0, 5), (5, 8))
# accumulating ops: (engine V/A, kind xx/xg/gg, chunk_start, chunk_end)
OPS = (
    ("A", "xx", 0, 8),
    ("V", "xg", 0, 5),
    ("V", "gg", 4, 5),
    ("V", "xg", 5, 8),
    ("A", "gg", 0, 4),
    ("A", "gg", 5, 8),
)
# packed idx tensor layout (int32 columns):
#   [0, 20)  piece-0 wrapped int16 idx (40 i16 cols)
#   [20, 32) piece-1 wrapped int16 idx (24 i16 cols)
#   [32, 36) scatter identity idx (8 i16 cols)
SW_COLS = {0: (0, 20), 1: (20, 12)}
SC2 = 32
IDXW = 36
OC = 64                   # scatter elem = 64 f32 = 256B (hardware minimum)

_CACHE: dict = {}


def _strip_barriers(nc):
    """Remove the framework's start/end all-engine barriers and drains.

    Every cross-engine dependency in this program flows through DMA or
    engine semaphores, so the barriers only serialize the launch/tail.
    """
    fn = nc.m.functions[0]
    for bb in fn.blocks:
        il = bb.instructions
        for inst in list(il):
            tn = type(inst).__name__
            if tn == "InstEventSemaphore" and inst.name.startswith("barrier_"):
                il.remove(inst)
            elif tn == "InstDrain":
                il.remove(inst)


def _reload_early(nc):
    """Move the Q7 ucode library-reload (95ns) ahead of the idx-DMA wait
    so it runs during the otherwise idle launch window."""
    fn = nc.m.functions[0]
    for bb in fn.blocks:
        il = bb.instructions
        pool = [i for i in il
                if str(getattr(i, "engine", "")) == "EngineType.Pool"]
        reloads = [i for i in pool
                   if type(i).__name__ == "InstPseudoReloadLibraryIndex"]
        if not reloads:
            continue
        first_wait = None
        for i in pool:
            si = i.sync_info
            if si is not None and len(si.wait_conditions) > 0:
                first_wait = i
                break
        if first_wait is None:
            continue
        for r in reloads:
            il.remove(r)
        pos = il.index(first_wait)
        for j, r in enumerate(reloads):
            il.insert(pos + j, r)


def _build():
    import concourse.bacc as bacc
    import concourse.bass as bass
    import concourse.mybir as mybir

    nc = bacc.Bacc(
        "TRN2",
        target_bir_lowering=False,
        debug=False,
        enable_asserts=False,
        num_devices=NCORES,
    )
    f32 = mybir.dt.float32
    x_d = nc.dram_tensor("x", [NS, D], f32, kind="ExternalInput")
    c_d = nc.dram_tensor("center", [M, D], f32, kind="ExternalInput")
    i_d = nc.dram_tensor("idx", [128, IDXW], mybir.dt.int32, kind="ExternalInput")
    o_d = nc.dram_tensor("out", [128, OC], f32, kind="ExternalOutput")
    nred = len(OPS)

    with (
        nc.sbuf_tensor("x_t", [128, C, D], f32) as x_t,
        nc.sbuf_tensor("g_t", [128, C, D], f32) as g_t,
        nc.sbuf_tensor("tmpv", [128, C, D], f32) as tmpv,
        nc.sbuf_tensor("tmpa", [128, C, D], f32) as tmpa,
        nc.sbuf_tensor("idx_t", [128, IDXW], mybir.dt.int32) as idx_t,
        nc.sbuf_tensor("obuf", [128, OC], f32) as obuf,
        nc.sbuf_tensor("zbias", [128, 1], f32) as zbias,
        nc.sbuf_tensor("zdum", [128, 1], f32) as zdum,
        nc.semaphore("s_idx") as s_idx,
        nc.semaphore("s_z") as s_z,
        nc.semaphore("s_x") as s_x,
        nc.semaphore("s_g0") as s_g0,
        nc.semaphore("s_g1") as s_g1,
        nc.semaphore("s_prep") as s_prep,
        nc.semaphore("s_red") as s_red,
        nc.semaphore("s_out") as s_out,
        nc.Block() as block,
    ):
        s_g = [s_g0, s_g1]

        @block.sync
        def _(sync: "bass.BassSync"):
            # idx first: it gates the Q7 descriptor generation
            sync.dma_start(idx_t[:], i_d.ap()).then_inc(s_idx, 16)
            # x as one contiguous DMA; slot (q, c) = row q*C + c
            x_src = x_d.ap().rearrange("(q c) d -> q c d", q=128)
            sync.dma_start(x_t[:], x_src).then_inc(s_x, 16)

        @block.gpsimd
        def _(gpsimd: "bass.BassGpSimd"):
            gpsimd.wait_ge(s_idx, 16)
            # dispatch both gather preps back-to-back so the Q7 generates
            # descriptors without waiting on the first trigger, then fire
            # each transfer as its generation commits
            for i, (c0, c1) in enumerate(PIECES):
                sc, ncol32 = SW_COLS[i]
                n_idx = (c1 - c0) * 128
                idx16 = idx_t[:, sc:sc + ncol32].bitcast(mybir.dt.int16)
                gpsimd.dma_gather(
                    g_t[:, c0:c1, :], c_d.ap(), idx16,
                    n_idx, n_idx, D,
                    prepare_only=True, sem=s_g[i],
                ).then_inc(s_prep, 1)
            for i in range(len(PIECES)):
                gpsimd.wait_ge(s_prep, i + 1)
                gpsimd.trigger_dma(count=1)
            gpsimd.dma_scatter_add(
                o_d.ap(),
                obuf[:].rearrange("q (a e) -> q a e", a=1),
                idx_t[:, SC2:SC2 + 4].bitcast(mybir.dt.int16),
                128, 128, OC,
                prepare_only=True, sem=s_out,
            ).then_inc(s_prep, 1)
            gpsimd.wait_ge(s_prep, len(PIECES) + 1)
            gpsimd.wait_ge(s_red, nred)
            gpsimd.trigger_dma(count=1)

        def emit_op(eng, is_act, kind, c0, c1, colidx, tmp):
            xs = x_t[:, c0:c1, :]
            gs = g_t[:, c0:c1, :]
            if kind in ("xx", "xg"):
                eng.wait_ge(s_x, 16)
            if kind in ("gg", "xg"):
                for i, (p0, p1) in enumerate(PIECES):
                    if c0 < p1 and c1 > p0:
                        eng.wait_ge(s_g[i], 16)
            if is_act:
                src = gs if kind == "gg" else xs
                eng.activation(
                    tmp[:, c0:c1, :], src,
                    mybir.ActivationFunctionType.Square,
                    bias=zbias[:],
                    accum_out=obuf[:, colidx:colidx + 1],
                ).then_inc(s_red, 1)
            else:
                if kind == "xg":
                    in0, in1, scal = xs, gs, -2.0
                elif kind == "gg":
                    in0, in1, scal = gs, gs, 1.0
                else:
                    in0, in1, scal = xs, xs, 1.0
                eng.scalar_tensor_tensor(
                    tmp[:, c0:c1, :], in0, scal, in1,
                    mybir.AluOpType.mult, mybir.AluOpType.mult,
                    accum_out=obuf[:, colidx:colidx + 1],
                ).then_inc(s_red, 1)

        @block.vector
        def _(vector: "bass.BassVector"):
            vector.memset(zbias[:], 0.0).then_inc(s_z, 1)
            vector.memset(obuf[:], 0.0).then_inc(s_z, 1)
            for i, (e, kind, c0, c1) in enumerate(OPS):
                if e == "V":
                    emit_op(vector, False, kind, c0, c1, i, tmpv)

        @block.scalar
        def _(scalar: "bass.BassScalar"):
            scalar.wait_ge(s_z, 2)
            # tiny dummy activation: forces the Square table load off the
            # critical path (it would otherwise run right before the first
            # real activation, after its data waits)
            scalar.activation(
                zdum[:], zbias[:],
                mybir.ActivationFunctionType.Square, bias=zbias[:])
            for i, (e, kind, c0, c1) in enumerate(OPS):
                if e == "A":
                    emit_op(scalar, True, kind, c0, c1, i, tmpa)

    _strip_barriers(nc)
    _reload_early(nc)
    nc.compile()
    return nc


def _get_nc():
    if "nc" not in _CACHE:
        _CACHE["nc"] = _build()
    return _CACHE["nc"]


def make_in_maps(inputs: np.ndarray, center: np.ndarray, labels: np.ndarray):
    """Shard full inputs into per-core input maps."""
    x = np.ascontiguousarray(np.asarray(inputs, dtype=np.float32))
    cen = np.ascontiguousarray(np.asarray(center, dtype=np.float32))
    lab = np.asarray(labels).astype(np.int32)
    # scatter identity, wrapped in 16 partitions and replicated x8
    wi = np.arange(128, dtype=np.int16).reshape(8, 16).T          # [16, 8]
    wi128 = np.ascontiguousarray(np.tile(wi, (8, 1))).view(np.int32)
    in_maps = []
    for k in range(NCORES):
        lab_k = lab[k * NS:(k + 1) * NS]
        offs = lab_k.reshape(128, C)          # offs[q, c] = label of row q*C+c
        idx = np.zeros((128, IDXW), dtype=np.int32)
        for i, (c0, c1) in enumerate(PIECES):
            sc, ncol32 = SW_COLS[i]
            n = (c1 - c0) * 128
            # gather element j -> slot (q=j%128, c=c0+j//128); wrapped int16
            # layout: element j at [j%16 (+16g replicas), j//16]
            j = np.arange(n)
            vals = offs[j % 128, c0 + j // 128].astype(np.int16)
            w = vals.reshape(-1, 16).T
            idx[:, sc:sc + ncol32] = np.ascontiguousarray(
                np.tile(w, (8, 1))).view(np.int32)
        idx[:, SC2:SC2 + 4] = wi128
        in_maps.append({
            "x": np.ascontiguousarray(x[k * NS:(k + 1) * NS]),
            "center": cen,
            "idx": idx,
        })
    return in_maps


def _run(in_maps):
    from concourse.bass_utils import run_bass_kernel_spmd

    nc = _get_nc()
    return run_bass_kernel_spmd(nc, in_maps, core_ids=list(range(NCORES)))


def kernel(inputs: np.ndarray, center: np.ndarray, labels: np.ndarray) -> np.ndarray:
    in_maps = make_in_maps(inputs, center, labels)
    res = _run(in_maps)
    # unshard: sum the per-core per-partition column partials, then the mean
    total = np.sum(
        np.stack([r["out"][:, :len(OPS)].astype(np.float32) for r in res.results]),
        dtype=np.float32,
    )
    return np.asarray(np.float32(total / np.float32(N)), dtype=np.float32)


if __name__ == "__main__":
    rng = np.random.default_rng(0)
    x = rng.standard_normal((N, D), dtype=np.float32)
    cen = rng.standard_normal((M, D), dtype=np.float32)
    lab = rng.integers(0, M, size=(N,), dtype=np.int64)
    got = kernel(x, cen, lab)
    sel = cen[lab]
    ref = np.mean(np.clip(np.sum((x - sel) ** 2, axis=1), 1e-12, 1e12))
    print("got", got, "ref", ref, "rel", abs(got - ref) / abs(ref))


# revision 6
# speedup vs baseline: 1.0167x; 1.0082x over previous
"""CenterLoss kernel for 8 TRN2 NeuronCores (raw Bass).

Computes mean_i ||x_i - center[labels_i]||^2 for x:[8192,128] f32,
center:[32000,128] f32, labels:[8192] int, via the decomposition

    sum ||x - g||^2 = sum x^2 - 2 sum x.g + sum g^2,   g_i = center[labels_i]

Strategy (data-parallel over the batch dim, per the sharding hint):
  - 8 cores, each takes a 1024-row shard of x/labels; the center table
    stays in HBM on every core and only the 1024 labeled rows are read,
    via SWDGE dma_gather in two pipelined pieces (5 + 3 chunks of 128
    rows), each prepared on the Q7 then trigger-fired so the transfer
    starts the moment its descriptor generation commits.
  - One packed idx DMA (wrapped int16 gather indices for both pieces +
    the replicated identity indices for the output scatter) so the Q7
    can start descriptor generation as early as possible.
  - Compute, one fused op per term: Act does Square-accumulate for
    sum x^2 (early, while gathers are in flight) and sum g^2 per piece;
    DVE scalar_tensor_tensor computes (-2x)*g with accum_out per piece.
    Each op lands in its own column of a [128,64]
    partial tile; no cross-engine combining on device.
  - Output via a prepared dma_scatter_add (identity indices) triggered
    once all six accumulating ops have signalled; the host sums the
    8 x 128 x 6 partials and divides by 8192 (the scalar all-reduce).
  - The framework's start/end all-engine barriers and drains are
    stripped post-build: every cross-engine data edge is ordered through
    DMA-completion or engine semaphores (Act's const-0 bias read is
    ordered behind the Pool-rooted gather-sem chain), so the barriers
    only add latency.

Validated exact (rel err ~1e-16 vs the fp32 reference recomputation)
across seeds on the axon execution path; TimelineSim 8258ns/core.

The kernel is self-contained: shapes are hardcoded below.
"""

import numpy as np

N, D, M = 8192, 128, 32000
NCORES = 8
NS = N // NCORES          # rows per core = 1024
C = NS // 128             # free-dim chunks per core = 8

# gather pieces: (chunk_start, chunk_end), all via SWDGE prepare+trigger
PIECES = ((0, 5), (5, 8))
# accumulating ops: (engine V/A, kind xx/xg/gg, chunk_start, chunk_end)
OPS = (
    ("A", "xx", 0, 8),
    ("V", "xg", 0, 5),
    ("V", "gg", 4, 5),
    ("V", "xg", 5, 8),
    ("A", "gg", 0, 4),
    ("A", "gg", 5, 8),
)
# packed idx tensor layout (int32 columns):
#   [0, 20)  piece-0 wrapped int16 idx (40 i16 cols)
#   [20, 32) piece-1 wrapped int16 idx (24 i16 cols)
#   [32, 36) scatter identity idx (8 i16 cols)
SW_COLS = {0: (0, 20), 1: (20, 12)}
SC2 = 32
IDXW = 36
OC = 64                   # scatter elem = 64 f32 = 256B (hardware minimum)

_CACHE: dict = {}


def _strip_barriers(nc):
    """Remove the framework's start/end all-engine barriers and drains.

    Every cross-engine dependency in this program flows through DMA or
    engine semaphores, so the barriers only serialize the launch/tail.
    """
    fn = nc.m.functions[0]
    for bb in fn.blocks:
        il = bb.instructions
        for inst in list(il):
            tn = type(inst).__name__
            if tn == "InstEventSemaphore" and inst.name.startswith("barrier_"):
                il.remove(inst)
            elif tn == "InstDrain":
                il.remove(inst)


def _reload_early(nc):
    """Move the Q7 ucode library-reload (95ns) ahead of the idx-DMA wait
    so it runs during the otherwise idle launch window."""
    fn = nc.m.functions[0]
    for bb in fn.blocks:
        il = bb.instructions
        pool = [i for i in il
                if str(getattr(i, "engine", "")) == "EngineType.Pool"]
        reloads = [i for i in pool
                   if type(i).__name__ == "InstPseudoReloadLibraryIndex"]
        if not reloads:
            continue
        first_wait = None
        for i in pool:
            si = i.sync_info
            if si is not None and len(si.wait_conditions) > 0:
                first_wait = i
                break
        if first_wait is None:
            continue
        for r in reloads:
            il.remove(r)
        pos = il.index(first_wait)
        for j, r in enumerate(reloads):
            il.insert(pos + j, r)


def _build():
    import concourse.bacc as bacc
    import concourse.bass as bass
    import concourse.mybir as mybir

    nc = bacc.Bacc(
        "TRN2",
        target_bir_lowering=False,
        debug=False,
        enable_asserts=False,
        num_devices=NCORES,
    )
    f32 = mybir.dt.float32
    x_d = nc.dram_tensor("x", [NS, D], f32, kind="ExternalInput")
    c_d = nc.dram_tensor("center", [M, D], f32, kind="ExternalInput")
    i_d = nc.dram_tensor("idx", [128, IDXW], mybir.dt.int32, kind="ExternalInput")
    o_d = nc.dram_tensor("out", [128, OC], f32, kind="ExternalOutput")
    nred = len(OPS)

    with (
        nc.sbuf_tensor("x_t", [128, C, D], f32) as x_t,
        nc.sbuf_tensor("g_t", [128, C, D], f32) as g_t,
        nc.sbuf_tensor("tmpv", [128, C, D], f32) as tmpv,
        nc.sbuf_tensor("tmpa", [128, C, D], f32) as tmpa,
        nc.sbuf_tensor("idx_t", [128, IDXW], mybir.dt.int32) as idx_t,
        nc.sbuf_tensor("obuf", [128, OC], f32) as obuf,
        nc.sbuf_tensor("zbias", [128, 1], f32) as zbias,
        nc.sbuf_tensor("zdum", [128, 1], f32) as zdum,
        nc.semaphore("s_idx") as s_idx,
        nc.semaphore("s_z") as s_z,
        nc.semaphore("s_x") as s_x,
        nc.semaphore("s_g0") as s_g0,
        nc.semaphore("s_g1") as s_g1,
        nc.semaphore("s_prep") as s_prep,
        nc.semaphore("s_red") as s_red,
        nc.semaphore("s_out") as s_out,
        nc.Block() as block,
    ):
        s_g = [s_g0, s_g1]

        @block.sync
        def _(sync: "bass.BassSync"):
            # idx first: it gates the Q7 descriptor generation
            sync.dma_start(idx_t[:], i_d.ap()).then_inc(s_idx, 16)
            # x as one contiguous DMA; slot (q, c) = row q*C + c
            x_src = x_d.ap().rearrange("(q c) d -> q c d", q=128)
            sync.dma_start(x_t[:], x_src).then_inc(s_x, 16)

        @block.gpsimd
        def _(gpsimd: "bass.BassGpSimd"):
            # pre-load the num_idxs registers so their RegisterMoves run in
            # the idle window before the idx DMA lands, not after the wait
            pre_regs = [gpsimd.to_reg((c1 - c0) * 128) for (c0, c1) in PIECES]
            gpsimd.wait_ge(s_idx, 16)
            # dispatch both gather preps back-to-back so the Q7 generates
            # descriptors without waiting on the first trigger, then fire
            # each transfer as its generation commits
            for i, (c0, c1) in enumerate(PIECES):
                sc, ncol32 = SW_COLS[i]
                n_idx = (c1 - c0) * 128
                idx16 = idx_t[:, sc:sc + ncol32].bitcast(mybir.dt.int16)
                gpsimd.dma_gather(
                    g_t[:, c0:c1, :], c_d.ap(), idx16,
                    n_idx, pre_regs[i], D,
                    prepare_only=True, sem=s_g[i],
                ).then_inc(s_prep, 1)
            for i in range(len(PIECES)):
                gpsimd.wait_ge(s_prep, i + 1)
                gpsimd.trigger_dma(count=1)
            gpsimd.dma_scatter_add(
                o_d.ap(),
                obuf[:].rearrange("q (a e) -> q a e", a=1),
                idx_t[:, SC2:SC2 + 4].bitcast(mybir.dt.int16),
                128, 128, OC,
                prepare_only=True, sem=s_out,
            ).then_inc(s_prep, 1)
            gpsimd.wait_ge(s_prep, len(PIECES) + 1)
            gpsimd.wait_ge(s_red, nred)
            gpsimd.trigger_dma(count=1)

        def emit_op(eng, is_act, kind, c0, c1, colidx, tmp):
            xs = x_t[:, c0:c1, :]
            gs = g_t[:, c0:c1, :]
            if kind in ("xx", "xg"):
                eng.wait_ge(s_x, 16)
            if kind in ("gg", "xg"):
                for i, (p0, p1) in enumerate(PIECES):
                    if c0 < p1 and c1 > p0:
                        eng.wait_ge(s_g[i], 16)
            if is_act:
                src = gs if kind == "gg" else xs
                eng.activation(
                    tmp[:, c0:c1, :], src,
                    mybir.ActivationFunctionType.Square,
                    bias=zbias[:],
                    accum_out=obuf[:, colidx:colidx + 1],
                ).then_inc(s_red, 1)
            else:
                if kind == "xg":
                    in0, in1, scal = xs, gs, -2.0
                elif kind == "gg":
                    in0, in1, scal = gs, gs, 1.0
                else:
                    in0, in1, scal = xs, xs, 1.0
                eng.scalar_tensor_tensor(
                    tmp[:, c0:c1, :], in0, scal, in1,
                    mybir.AluOpType.mult, mybir.AluOpType.mult,
                    accum_out=obuf[:, colidx:colidx + 1],
                ).then_inc(s_red, 1)

        @block.vector
        def _(vector: "bass.BassVector"):
            vector.memset(zbias[:], 0.0).then_inc(s_z, 1)
            vector.memset(obuf[:], 0.0).then_inc(s_z, 1)
            for i, (e, kind, c0, c1) in enumerate(OPS):
                if e == "V":
                    emit_op(vector, False, kind, c0, c1, i, tmpv)

        @block.scalar
        def _(scalar: "bass.BassScalar"):
            scalar.wait_ge(s_z, 2)
            # tiny dummy activation: forces the Square table load off the
            # critical path (it would otherwise run right before the first
            # real activation, after its data waits)
            scalar.activation(
                zdum[:], zbias[:],
                mybir.ActivationFunctionType.Square, bias=zbias[:])
            for i, (e, kind, c0, c1) in enumerate(OPS):
                if e == "A":
                    emit_op(scalar, True, kind, c0, c1, i, tmpa)

    _strip_barriers(nc)
    _reload_early(nc)
    nc.compile()
    return nc


def _get_nc():
    if "nc" not in _CACHE:
        _CACHE["nc"] = _build()
    return _CACHE["nc"]


def make_in_maps(inputs: np.ndarray, center: np.ndarray, labels: np.ndarray):
    """Shard full inputs into per-core input maps."""
    x = np.ascontiguousarray(np.asarray(inputs, dtype=np.float32))
    cen = np.ascontiguousarray(np.asarray(center, dtype=np.float32))
    lab = np.asarray(labels).astype(np.int32)
    # scatter identity, wrapped in 16 partitions and replicated x8
    wi = np.arange(128, dtype=np.int16).reshape(8, 16).T          # [16, 8]
    wi128 = np.ascontiguousarray(np.tile(wi, (8, 1))).view(np.int32)
    in_maps = []
    for k in range(NCORES):
        lab_k = lab[k * NS:(k + 1) * NS]
        offs = lab_k.reshape(128, C)          # offs[q, c] = label of row q*C+c
        idx = np.zeros((128, IDXW), dtype=np.int32)
        for i, (c0, c1) in enumerate(PIECES):
            sc, ncol32 = SW_COLS[i]
            n = (c1 - c0) * 128
            # gather element j -> slot (q=j%128, c=c0+j//128); wrapped int16
            # layout: element j at [j%16 (+16g replicas), j//16]
            j = np.arange(n)
            vals = offs[j % 128, c0 + j // 128].astype(np.int16)
            w = vals.reshape(-1, 16).T
            idx[:, sc:sc + ncol32] = np.ascontiguousarray(
                np.tile(w, (8, 1))).view(np.int32)
        idx[:, SC2:SC2 + 4] = wi128
        in_maps.append({
            "x": np.ascontiguousarray(x[k * NS:(k + 1) * NS]),
            "center": cen,
            "idx": idx,
        })
    return in_maps


def _run(in_maps):
    from concourse.bass_utils import run_bass_kernel_spmd

    nc = _get_nc()
    return run_bass_kernel_spmd(nc, in_maps, core_ids=list(range(NCORES)))


def kernel(inputs: np.ndarray, center: np.ndarray, labels: np.ndarray) -> np.ndarray:
    in_maps = make_in_maps(inputs, center, labels)
    res = _run(in_maps)
    # unshard: sum the per-core per-partition column partials, then the mean
    total = np.sum(
        np.stack([r["out"][:, :len(OPS)].astype(np.float32) for r in res.results]),
        dtype=np.float32,
    )
    return np.asarray(np.float32(total / np.float32(N)), dtype=np.float32)


if __name__ == "__main__":
    rng = np.random.default_rng(0)
    x = rng.standard_normal((N, D), dtype=np.float32)
    cen = rng.standard_normal((M, D), dtype=np.float32)
    lab = rng.integers(0, M, size=(N,), dtype=np.int64)
    got = kernel(x, cen, lab)
    sel = cen[lab]
    ref = np.mean(np.clip(np.sum((x - sel) ** 2, axis=1), 1e-12, 1e12))
    print("got", got, "ref", ref, "rel", abs(got - ref) / abs(ref))


# revision 7
# speedup vs baseline: 1.0277x; 1.0108x over previous
"""CenterLoss kernel for 8 TRN2 NeuronCores (raw Bass).

Computes mean_i ||x_i - center[labels_i]||^2 for x:[8192,128] f32,
center:[32000,128] f32, labels:[8192] int, via the decomposition

    sum ||x - g||^2 = sum x^2 - 2 sum x.g + sum g^2,   g_i = center[labels_i]

Strategy (data-parallel over the batch dim, per the sharding hint):
  - 8 cores, each takes a 1024-row shard of x/labels; the center table
    stays in HBM on every core and only the 1024 labeled rows are read,
    via SWDGE dma_gather in two pipelined pieces (5 + 3 chunks of 128
    rows), each prepared on the Q7 then trigger-fired so the transfer
    starts the moment its descriptor generation commits.
  - One packed idx DMA (wrapped int16 gather indices for both pieces +
    the replicated identity indices for the output scatter) so the Q7
    can start descriptor generation as early as possible.
  - Compute, one fused op per term: DVE scalar_tensor_tensor computes
    sum x^2 (early, in DVE's idle window while gathers are in flight)
    and (-2x)*g with accum_out per piece; Act does Square-accumulate
    for sum g^2, split 4/1/3 across Act/DVE/Act so both engines' chains
    finish together. Each op lands in its own column of a [128,64]
    partial tile; no cross-engine combining on device.
  - Output via a prepared dma_scatter_add (identity indices) triggered
    once all six accumulating ops have signalled; the host sums the
    8 x 128 x 6 partials and divides by 8192 (the scalar all-reduce).
  - The framework's start/end all-engine barriers and drains are
    stripped post-build: every cross-engine data edge is ordered through
    DMA-completion or engine semaphores (Act's const-0 bias read is
    ordered behind the Pool-rooted gather-sem chain), so the barriers
    only add latency.

Validated exact (rel err ~1e-16 vs the fp32 reference recomputation)
across seeds on the axon execution path; TimelineSim 8193ns/core.

The kernel is self-contained: shapes are hardcoded below.
"""

import numpy as np

N, D, M = 8192, 128, 32000
NCORES = 8
NS = N // NCORES          # rows per core = 1024
C = NS // 128             # free-dim chunks per core = 8

# gather pieces: (chunk_start, chunk_end), all via SWDGE prepare+trigger
PIECES = ((0, 5), (5, 8))
# accumulating ops: (engine V/A, kind xx/xg/gg, chunk_start, chunk_end)
OPS = (
    ("V", "xx", 0, 8),
    ("V", "xg", 0, 5),
    ("V", "gg", 4, 5),
    ("V", "xg", 5, 8),
    ("A", "gg", 0, 4),
    ("A", "gg", 5, 8),
)
# packed idx tensor layout (int32 columns):
#   [0, 20)  piece-0 wrapped int16 idx (40 i16 cols)
#   [20, 32) piece-1 wrapped int16 idx (24 i16 cols)
#   [32, 36) scatter identity idx (8 i16 cols)
SW_COLS = {0: (0, 20), 1: (20, 12)}
SC2 = 32
IDXW = 36
OC = 64                   # scatter elem = 64 f32 = 256B (hardware minimum)

_CACHE: dict = {}


def _strip_barriers(nc):
    """Remove the framework's start/end all-engine barriers and drains.

    Every cross-engine dependency in this program flows through DMA or
    engine semaphores, so the barriers only serialize the launch/tail.
    """
    fn = nc.m.functions[0]
    for bb in fn.blocks:
        il = bb.instructions
        for inst in list(il):
            tn = type(inst).__name__
            if tn == "InstEventSemaphore" and inst.name.startswith("barrier_"):
                il.remove(inst)
            elif tn == "InstDrain":
                il.remove(inst)


def _reload_early(nc):
    """Move the Q7 ucode library-reload (95ns) ahead of the idx-DMA wait
    so it runs during the otherwise idle launch window."""
    fn = nc.m.functions[0]
    for bb in fn.blocks:
        il = bb.instructions
        pool = [i for i in il
                if str(getattr(i, "engine", "")) == "EngineType.Pool"]
        reloads = [i for i in pool
                   if type(i).__name__ == "InstPseudoReloadLibraryIndex"]
        if not reloads:
            continue
        first_wait = None
        for i in pool:
            si = i.sync_info
            if si is not None and len(si.wait_conditions) > 0:
                first_wait = i
                break
        if first_wait is None:
            continue
        for r in reloads:
            il.remove(r)
        pos = il.index(first_wait)
        for j, r in enumerate(reloads):
            il.insert(pos + j, r)


def _build():
    import concourse.bacc as bacc
    import concourse.bass as bass
    import concourse.mybir as mybir

    nc = bacc.Bacc(
        "TRN2",
        target_bir_lowering=False,
        debug=False,
        enable_asserts=False,
        num_devices=NCORES,
    )
    f32 = mybir.dt.float32
    x_d = nc.dram_tensor("x", [NS, D], f32, kind="ExternalInput")
    c_d = nc.dram_tensor("center", [M, D], f32, kind="ExternalInput")
    i_d = nc.dram_tensor("idx", [128, IDXW], mybir.dt.int32, kind="ExternalInput")
    o_d = nc.dram_tensor("out", [128, OC], f32, kind="ExternalOutput")
    nred = len(OPS)

    with (
        nc.sbuf_tensor("x_t", [128, C, D], f32) as x_t,
        nc.sbuf_tensor("g_t", [128, C, D], f32) as g_t,
        nc.sbuf_tensor("tmpv", [128, C, D], f32) as tmpv,
        nc.sbuf_tensor("tmpa", [128, C, D], f32) as tmpa,
        nc.sbuf_tensor("idx_t", [128, IDXW], mybir.dt.int32) as idx_t,
        nc.sbuf_tensor("obuf", [128, OC], f32) as obuf,
        nc.sbuf_tensor("zbias", [128, 1], f32) as zbias,
        nc.sbuf_tensor("zdum", [128, 1], f32) as zdum,
        nc.semaphore("s_idx") as s_idx,
        nc.semaphore("s_z") as s_z,
        nc.semaphore("s_x") as s_x,
        nc.semaphore("s_g0") as s_g0,
        nc.semaphore("s_g1") as s_g1,
        nc.semaphore("s_prep") as s_prep,
        nc.semaphore("s_red") as s_red,
        nc.semaphore("s_out") as s_out,
        nc.Block() as block,
    ):
        s_g = [s_g0, s_g1]

        @block.sync
        def _(sync: "bass.BassSync"):
            # idx first: it gates the Q7 descriptor generation
            sync.dma_start(idx_t[:], i_d.ap()).then_inc(s_idx, 16)
            # x as one contiguous DMA; slot (q, c) = row q*C + c
            x_src = x_d.ap().rearrange("(q c) d -> q c d", q=128)
            sync.dma_start(x_t[:], x_src).then_inc(s_x, 16)

        @block.gpsimd
        def _(gpsimd: "bass.BassGpSimd"):
            # pre-load the num_idxs registers so their RegisterMoves run in
            # the idle window before the idx DMA lands, not after the wait
            pre_regs = [gpsimd.to_reg((c1 - c0) * 128) for (c0, c1) in PIECES]
            gpsimd.wait_ge(s_idx, 16)
            # dispatch both gather preps back-to-back so the Q7 generates
            # descriptors without waiting on the first trigger, then fire
            # each transfer as its generation commits
            for i, (c0, c1) in enumerate(PIECES):
                sc, ncol32 = SW_COLS[i]
                n_idx = (c1 - c0) * 128
                idx16 = idx_t[:, sc:sc + ncol32].bitcast(mybir.dt.int16)
                gpsimd.dma_gather(
                    g_t[:, c0:c1, :], c_d.ap(), idx16,
                    n_idx, pre_regs[i], D,
                    prepare_only=True, sem=s_g[i],
                ).then_inc(s_prep, 1)
            for i in range(len(PIECES)):
                gpsimd.wait_ge(s_prep, i + 1)
                gpsimd.trigger_dma(count=1)
            gpsimd.dma_scatter_add(
                o_d.ap(),
                obuf[:].rearrange("q (a e) -> q a e", a=1),
                idx_t[:, SC2:SC2 + 4].bitcast(mybir.dt.int16),
                128, 128, OC,
                prepare_only=True, sem=s_out,
            ).then_inc(s_prep, 1)
            gpsimd.wait_ge(s_prep, len(PIECES) + 1)
            gpsimd.wait_ge(s_red, nred)
            gpsimd.trigger_dma(count=1)

        def emit_op(eng, is_act, kind, c0, c1, colidx, tmp):
            xs = x_t[:, c0:c1, :]
            gs = g_t[:, c0:c1, :]
            if kind in ("xx", "xg"):
                eng.wait_ge(s_x, 16)
            if kind in ("gg", "xg"):
                for i, (p0, p1) in enumerate(PIECES):
                    if c0 < p1 and c1 > p0:
                        eng.wait_ge(s_g[i], 16)
            if is_act:
                src = gs if kind == "gg" else xs
                eng.activation(
                    tmp[:, c0:c1, :], src,
                    mybir.ActivationFunctionType.Square,
                    bias=zbias[:],
                    accum_out=obuf[:, colidx:colidx + 1],
                ).then_inc(s_red, 1)
            else:
                if kind == "xg":
                    in0, in1, scal = xs, gs, -2.0
                elif kind == "gg":
                    in0, in1, scal = gs, gs, 1.0
                else:
                    in0, in1, scal = xs, xs, 1.0
                eng.scalar_tensor_tensor(
                    tmp[:, c0:c1, :], in0, scal, in1,
                    mybir.AluOpType.mult, mybir.AluOpType.mult,
                    accum_out=obuf[:, colidx:colidx + 1],
                ).then_inc(s_red, 1)

        @block.vector
        def _(vector: "bass.BassVector"):
            vector.memset(zbias[:], 0.0).then_inc(s_z, 1)
            vector.memset(obuf[:], 0.0).then_inc(s_z, 1)
            for i, (e, kind, c0, c1) in enumerate(OPS):
                if e == "V":
                    emit_op(vector, False, kind, c0, c1, i, tmpv)

        @block.scalar
        def _(scalar: "bass.BassScalar"):
            scalar.wait_ge(s_z, 2)
            # tiny dummy activation: forces the Square table load off the
            # critical path (it would otherwise run right before the first
            # real activation, after its data waits)
            scalar.activation(
                zdum[:], zbias[:],
                mybir.ActivationFunctionType.Square, bias=zbias[:])
            for i, (e, kind, c0, c1) in enumerate(OPS):
                if e == "A":
                    emit_op(scalar, True, kind, c0, c1, i, tmpa)

    _strip_barriers(nc)
    _reload_early(nc)
    nc.compile()
    return nc


def _get_nc():
    if "nc" not in _CACHE:
        _CACHE["nc"] = _build()
    return _CACHE["nc"]


def make_in_maps(inputs: np.ndarray, center: np.ndarray, labels: np.ndarray):
    """Shard full inputs into per-core input maps."""
    x = np.ascontiguousarray(np.asarray(inputs, dtype=np.float32))
    cen = np.ascontiguousarray(np.asarray(center, dtype=np.float32))
    lab = np.asarray(labels).astype(np.int32)
    # scatter identity, wrapped in 16 partitions and replicated x8
    wi = np.arange(128, dtype=np.int16).reshape(8, 16).T          # [16, 8]
    wi128 = np.ascontiguousarray(np.tile(wi, (8, 1))).view(np.int32)
    in_maps = []
    for k in range(NCORES):
        lab_k = lab[k * NS:(k + 1) * NS]
        offs = lab_k.reshape(128, C)          # offs[q, c] = label of row q*C+c
        idx = np.zeros((128, IDXW), dtype=np.int32)
        for i, (c0, c1) in enumerate(PIECES):
            sc, ncol32 = SW_COLS[i]
            n = (c1 - c0) * 128
            # gather element j -> slot (q=j%128, c=c0+j//128); wrapped int16
            # layout: element j at [j%16 (+16g replicas), j//16]
            j = np.arange(n)
            vals = offs[j % 128, c0 + j // 128].astype(np.int16)
            w = vals.reshape(-1, 16).T
            idx[:, sc:sc + ncol32] = np.ascontiguousarray(
                np.tile(w, (8, 1))).view(np.int32)
        idx[:, SC2:SC2 + 4] = wi128
        in_maps.append({
            "x": np.ascontiguousarray(x[k * NS:(k + 1) * NS]),
            "center": cen,
            "idx": idx,
        })
    return in_maps


def _run(in_maps):
    from concourse.bass_utils import run_bass_kernel_spmd

    nc = _get_nc()
    return run_bass_kernel_spmd(nc, in_maps, core_ids=list(range(NCORES)))


def kernel(inputs: np.ndarray, center: np.ndarray, labels: np.ndarray) -> np.ndarray:
    in_maps = make_in_maps(inputs, center, labels)
    res = _run(in_maps)
    # unshard: sum the per-core per-partition column partials, then the mean
    total = np.sum(
        np.stack([r["out"][:, :len(OPS)].astype(np.float32) for r in res.results]),
        dtype=np.float32,
    )
    return np.asarray(np.float32(total / np.float32(N)), dtype=np.float32)


if __name__ == "__main__":
    rng = np.random.default_rng(0)
    x = rng.standard_normal((N, D), dtype=np.float32)
    cen = rng.standard_normal((M, D), dtype=np.float32)
    lab = rng.integers(0, M, size=(N,), dtype=np.int64)
    got = kernel(x, cen, lab)
    sel = cen[lab]
    ref = np.mean(np.clip(np.sum((x - sel) ** 2, axis=1), 1e-12, 1e12))
    print("got", got, "ref", ref, "rel", abs(got - ref) / abs(ref))


# revision 8
# speedup vs baseline: 1.0340x; 1.0062x over previous
"""CenterLoss kernel for 8 TRN2 NeuronCores (raw Bass).

Computes mean_i ||x_i - center[labels_i]||^2 for x:[8192,128] f32,
center:[32000,128] f32, labels:[8192] int, via the decomposition

    sum ||x - g||^2 = sum x^2 - 2 sum x.g + sum g^2,   g_i = center[labels_i]

Strategy (data-parallel over the batch dim, per the sharding hint):
  - 8 cores, each takes a 1024-row shard of x/labels; the center table
    stays in HBM on every core and only the 1024 labeled rows are read,
    via SWDGE dma_gather in two pipelined pieces (5 + 3 chunks of 128
    rows), each prepared on the Q7 then trigger-fired so the transfer
    starts the moment its descriptor generation commits.
  - One packed idx DMA (wrapped int16 gather indices for both pieces +
    the replicated identity indices for the output scatter) so the Q7
    can start descriptor generation as early as possible.
  - Compute, one fused op per term: DVE scalar_tensor_tensor computes
    sum x^2 (early, in DVE's idle window while gathers are in flight)
    and (-2x)*g with accum_out per piece; Act does Square-accumulate
    for sum g^2, split 4/1/3 across Act/DVE/Act so both engines' chains
    finish together. Each op lands in its own column of a [128,64]
    partial tile; no cross-engine combining on device.
  - Output via a prepared dma_scatter_add (identity indices) triggered
    once all six accumulating ops have signalled; the host sums the
    8 x 128 x 6 partials and divides by 8192 (the scalar all-reduce).
  - The framework's start/end all-engine barriers and drains are
    stripped post-build: every cross-engine data edge is ordered through
    DMA-completion or engine semaphores (Act's const-0 bias read is
    ordered behind the Pool-rooted gather-sem chain), so the barriers
    only add latency.

Validated exact (rel err ~1e-16 vs the fp32 reference recomputation)
across seeds on the axon execution path; TimelineSim 8143ns/core.

The kernel is self-contained: shapes are hardcoded below.
"""

import numpy as np

N, D, M = 8192, 128, 32000
NCORES = 8
NS = N // NCORES          # rows per core = 1024
C = NS // 128             # free-dim chunks per core = 8

# gather pieces: (chunk_start, chunk_end), all via SWDGE prepare+trigger
PIECES = ((0, 5), (5, 8))
# accumulating ops: (engine V/A, kind xx/xg/gg, chunk_start, chunk_end)
OPS = (
    ("V", "xx", 0, 8),
    ("V", "xg", 0, 5),
    ("V", "gg", 4, 5),
    ("V", "xg", 5, 8),
    ("A", "gg", 0, 4),
    ("A", "gg", 5, 8),
)
# packed idx tensor layout (int32 columns):
#   [0, 20)  piece-0 wrapped int16 idx (40 i16 cols)
#   [20, 32) piece-1 wrapped int16 idx (24 i16 cols)
#   [32, 36) scatter identity idx (8 i16 cols)
SW_COLS = {0: (0, 20), 1: (20, 12)}
SC2 = 32
IDXW = 36
OC = 64                   # scatter elem = 64 f32 = 256B (hardware minimum)

_CACHE: dict = {}


def _strip_barriers(nc):
    """Remove the framework's start/end all-engine barriers and drains.

    Every cross-engine dependency in this program flows through DMA or
    engine semaphores, so the barriers only serialize the launch/tail.
    """
    fn = nc.m.functions[0]
    for bb in fn.blocks:
        il = bb.instructions
        for inst in list(il):
            tn = type(inst).__name__
            if tn == "InstEventSemaphore" and inst.name.startswith("barrier_"):
                il.remove(inst)
            elif tn in ("InstDrain", "InstUnconditionalBranch"):
                il.remove(inst)


def _reload_early(nc):
    """Move the Q7 ucode library-reload (95ns) ahead of the idx-DMA wait
    so it runs during the otherwise idle launch window."""
    fn = nc.m.functions[0]
    for bb in fn.blocks:
        il = bb.instructions
        pool = [i for i in il
                if str(getattr(i, "engine", "")) == "EngineType.Pool"]
        reloads = [i for i in pool
                   if type(i).__name__ == "InstPseudoReloadLibraryIndex"]
        if not reloads:
            continue
        first_wait = None
        for i in pool:
            si = i.sync_info
            if si is not None and len(si.wait_conditions) > 0:
                first_wait = i
                break
        if first_wait is None:
            continue
        for r in reloads:
            il.remove(r)
        pos = il.index(first_wait)
        for j, r in enumerate(reloads):
            il.insert(pos + j, r)


def _build():
    import concourse.bacc as bacc
    import concourse.bass as bass
    import concourse.mybir as mybir

    nc = bacc.Bacc(
        "TRN2",
        target_bir_lowering=False,
        debug=False,
        enable_asserts=False,
        num_devices=NCORES,
    )
    f32 = mybir.dt.float32
    x_d = nc.dram_tensor("x", [NS, D], f32, kind="ExternalInput")
    c_d = nc.dram_tensor("center", [M, D], f32, kind="ExternalInput")
    i_d = nc.dram_tensor("idx", [128, IDXW], mybir.dt.int32, kind="ExternalInput")
    o_d = nc.dram_tensor("out", [128, OC], f32, kind="ExternalOutput")
    nred = len(OPS)

    with (
        nc.sbuf_tensor("x_t", [128, C, D], f32) as x_t,
        nc.sbuf_tensor("g_t", [128, C, D], f32) as g_t,
        nc.sbuf_tensor("tmpv", [128, C, D], f32) as tmpv,
        nc.sbuf_tensor("tmpa", [128, C, D], f32) as tmpa,
        nc.sbuf_tensor("idx_t", [128, IDXW], mybir.dt.int32) as idx_t,
        nc.sbuf_tensor("obuf", [128, OC], f32) as obuf,
        nc.sbuf_tensor("zbias", [128, 1], f32) as zbias,
        nc.sbuf_tensor("zdum", [128, 1], f32) as zdum,
        nc.semaphore("s_idx") as s_idx,
        nc.semaphore("s_z") as s_z,
        nc.semaphore("s_x") as s_x,
        nc.semaphore("s_g0") as s_g0,
        nc.semaphore("s_g1") as s_g1,
        nc.semaphore("s_prep") as s_prep,
        nc.semaphore("s_red") as s_red,
        nc.semaphore("s_out") as s_out,
        nc.Block() as block,
    ):
        s_g = [s_g0, s_g1]

        @block.sync
        def _(sync: "bass.BassSync"):
            # idx first: it gates the Q7 descriptor generation
            sync.dma_start(idx_t[:], i_d.ap()).then_inc(s_idx, 16)
            # x as one contiguous DMA; slot (q, c) = row q*C + c
            x_src = x_d.ap().rearrange("(q c) d -> q c d", q=128)
            sync.dma_start(x_t[:], x_src).then_inc(s_x, 16)

        @block.gpsimd
        def _(gpsimd: "bass.BassGpSimd"):
            # pre-load the num_idxs registers so their RegisterMoves run in
            # the idle window before the idx DMA lands, not after the wait
            pre_regs = [gpsimd.to_reg((c1 - c0) * 128) for (c0, c1) in PIECES]
            gpsimd.wait_ge(s_idx, 16)
            # dispatch both gather preps back-to-back so the Q7 generates
            # descriptors without waiting on the first trigger, then fire
            # each transfer as its generation commits
            for i, (c0, c1) in enumerate(PIECES):
                sc, ncol32 = SW_COLS[i]
                n_idx = (c1 - c0) * 128
                idx16 = idx_t[:, sc:sc + ncol32].bitcast(mybir.dt.int16)
                gpsimd.dma_gather(
                    g_t[:, c0:c1, :], c_d.ap(), idx16,
                    n_idx, pre_regs[i], D,
                    prepare_only=True, sem=s_g[i],
                ).then_inc(s_prep, 1)
            for i in range(len(PIECES)):
                gpsimd.wait_ge(s_prep, i + 1)
                gpsimd.trigger_dma(count=1)
            gpsimd.dma_scatter_add(
                o_d.ap(),
                obuf[:].rearrange("q (a e) -> q a e", a=1),
                idx_t[:, SC2:SC2 + 4].bitcast(mybir.dt.int16),
                128, 128, OC,
                prepare_only=True, sem=s_out,
            ).then_inc(s_prep, 1)
            gpsimd.wait_ge(s_prep, len(PIECES) + 1)
            gpsimd.wait_ge(s_red, nred)
            gpsimd.trigger_dma(count=1)

        def emit_op(eng, is_act, kind, c0, c1, colidx, tmp):
            xs = x_t[:, c0:c1, :]
            gs = g_t[:, c0:c1, :]
            if kind in ("xx", "xg"):
                eng.wait_ge(s_x, 16)
            if kind in ("gg", "xg"):
                for i, (p0, p1) in enumerate(PIECES):
                    if c0 < p1 and c1 > p0:
                        eng.wait_ge(s_g[i], 16)
            if is_act:
                src = gs if kind == "gg" else xs
                eng.activation(
                    tmp[:, c0:c1, :], src,
                    mybir.ActivationFunctionType.Square,
                    bias=zbias[:],
                    accum_out=obuf[:, colidx:colidx + 1],
                ).then_inc(s_red, 1)
            else:
                if kind == "xg":
                    in0, in1, scal = xs, gs, -2.0
                elif kind == "gg":
                    in0, in1, scal = gs, gs, 1.0
                else:
                    in0, in1, scal = xs, xs, 1.0
                eng.scalar_tensor_tensor(
                    tmp[:, c0:c1, :], in0, scal, in1,
                    mybir.AluOpType.mult, mybir.AluOpType.mult,
                    accum_out=obuf[:, colidx:colidx + 1],
                ).then_inc(s_red, 1)

        @block.vector
        def _(vector: "bass.BassVector"):
            vector.memset(zbias[:], 0.0).then_inc(s_z, 1)
            vector.memset(obuf[:], 0.0).then_inc(s_z, 1)
            for i, (e, kind, c0, c1) in enumerate(OPS):
                if e == "V":
                    emit_op(vector, False, kind, c0, c1, i, tmpv)

        @block.scalar
        def _(scalar: "bass.BassScalar"):
            scalar.wait_ge(s_z, 2)
            # tiny dummy activation: forces the Square table load off the
            # critical path (it would otherwise run right before the first
            # real activation, after its data waits)
            scalar.activation(
                zdum[:], zbias[:],
                mybir.ActivationFunctionType.Square, bias=zbias[:])
            for i, (e, kind, c0, c1) in enumerate(OPS):
                if e == "A":
                    emit_op(scalar, True, kind, c0, c1, i, tmpa)

    _strip_barriers(nc)
    _reload_early(nc)
    nc.compile()
    return nc


def _get_nc():
    if "nc" not in _CACHE:
        _CACHE["nc"] = _build()
    return _CACHE["nc"]


def make_in_maps(inputs: np.ndarray, center: np.ndarray, labels: np.ndarray):
    """Shard full inputs into per-core input maps."""
    x = np.ascontiguousarray(np.asarray(inputs, dtype=np.float32))
    cen = np.ascontiguousarray(np.asarray(center, dtype=np.float32))
    lab = np.asarray(labels).astype(np.int32)
    # scatter identity, wrapped in 16 partitions and replicated x8
    wi = np.arange(128, dtype=np.int16).reshape(8, 16).T          # [16, 8]
    wi128 = np.ascontiguousarray(np.tile(wi, (8, 1))).view(np.int32)
    in_maps = []
    for k in range(NCORES):
        lab_k = lab[k * NS:(k + 1) * NS]
        offs = lab_k.reshape(128, C)          # offs[q, c] = label of row q*C+c
        idx = np.zeros((128, IDXW), dtype=np.int32)
        for i, (c0, c1) in enumerate(PIECES):
            sc, ncol32 = SW_COLS[i]
            n = (c1 - c0) * 128
            # gather element j -> slot (q=j%128, c=c0+j//128); wrapped int16
            # layout: element j at [j%16 (+16g replicas), j//16]
            j = np.arange(n)
            vals = offs[j % 128, c0 + j // 128].astype(np.int16)
            w = vals.reshape(-1, 16).T
            idx[:, sc:sc + ncol32] = np.ascontiguousarray(
                np.tile(w, (8, 1))).view(np.int32)
        idx[:, SC2:SC2 + 4] = wi128
        in_maps.append({
            "x": np.ascontiguousarray(x[k * NS:(k + 1) * NS]),
            "center": cen,
            "idx": idx,
        })
    return in_maps


def _run(in_maps):
    from concourse.bass_utils import run_bass_kernel_spmd

    nc = _get_nc()
    return run_bass_kernel_spmd(nc, in_maps, core_ids=list(range(NCORES)))


def kernel(inputs: np.ndarray, center: np.ndarray, labels: np.ndarray) -> np.ndarray:
    in_maps = make_in_maps(inputs, center, labels)
    res = _run(in_maps)
    # unshard: sum the per-core per-partition column partials, then the mean
    total = np.sum(
        np.stack([r["out"][:, :len(OPS)].astype(np.float32) for r in res.results]),
        dtype=np.float32,
    )
    return np.asarray(np.float32(total / np.float32(N)), dtype=np.float32)


if __name__ == "__main__":
    rng = np.random.default_rng(0)
    x = rng.standard_normal((N, D), dtype=np.float32)
    cen = rng.standard_normal((M, D), dtype=np.float32)
    lab = rng.integers(0, M, size=(N,), dtype=np.int64)
    got = kernel(x, cen, lab)
    sel = cen[lab]
    ref = np.mean(np.clip(np.sum((x - sel) ** 2, axis=1), 1e-12, 1e12))
    print("got", got, "ref", ref, "rel", abs(got - ref) / abs(ref))
